# revision 17
# baseline (speedup 1.0000x reference)
"""BiAffine layer kernel for 8 Trainium2 NeuronCores.

Reference computation (per batch b):
  s = relu(x @ sW.T + sb)                  [L, E]
  t = relu(x @ tW.T + tb)                  [L, E]
  key = (s @ blW.T).reshape(L, E, N)
  out1[i, n, l] = sum_e key[i, e, n] * t[l, e]
  su = s @ Wu.T ; tv = t @ Wv.T            (Wu, Wv = f2W[:, :E], f2W[:, E:])
  h[i, j, :] = relu(su[i] + tv[j] + f2b)
  out2[i, n, j] = sum_e h[i, j, e] * f3W[n, e] + f3b[n]
  out = out1 + out2                        [L, N, L]

Sharding: 8 cores = 2 batches x 4 blocks of 128 source positions (i).

Octet layout: one PSUM bank [128, 512] holds EIGHT i's: 4 col-groups at
32-aligned offsets, 2 i's packed per group (rows 32k + 12s + n, 8 pad
rows per group).  out2 runs FIRST (initializes rows :24 of each group),
out1 accumulates after and stops the bank; the 8 pad rows per group are
never DMA'd so their garbage is harmless.  This lets the key-tensor prep
(48 matmuls + 24 copies) overlap the first octets' h production.

h production per octet, position p = i%8 (tile = (p, ec)):
  DVE: p 0-5 both ec -> h' = max(tv, -su') via TENSOR_SCALAR with a
    per-partition fp32 scalar pointer (2x DVE mode; the pointer-scalar
    occupies a read port so 4x is not reachable), dense fp16 [128, 512]
    tiles.  GPSIMD cannot help: its tensor_scalar measures ~8.4us/tile.
  ACT: p6/p7 both ec -> true h = relu(tv + su') via activation bias.
The h' tiles are short by sum_e f3W[n,e]su'[e,i]; that rank-1
correction C (p<6 rows only) is computed on the PE and folded into the
final copy's per-partition bias.
Final: one ACT copy [128,512] fp16 per octet (pipelined one octet back)
into a 4-octet output buffer; 4 sync-issued DMAs per 4 octets.

PE HAM warmup: ~10 throwaway matmuls issued during the input-DMA phase
so the PE clock is ramping to 2.4 GHz when real matmuls start.

DMA-instruction issue costs ~600ns+ (HWDGE; SWDGE multi-level patterns
cost much more - avoid), so all output DMAs use sync with plain
24-partition sources, and inputs load as few fp16 chunk-major DMAs.
"""

import sys

sys.path.insert(0, "/opt/trn_rl_repo")

import numpy as np

B, L, H, E, N = 2, 512, 768, 256, 12
EC = E // 128  # 2 e-chunks
HC = H // 128  # 6 h-chunks
IB = L // 4  # 128 i's per core
NCORES = 8
OCTS = IB // 8  # 16
OG = 4  # octets per output-DMA group

# misc fp32 tensor column layout: [sb(2) tb(2) f2b(2) f3b128(1) kxn01(16)
#                                  mask8(8) f3WT(24)]
MISC_W = 2 + 2 + 2 + 1 + OCTS + 8 + 2 * N

_cache = {}


def build_nc():
    import concourse.bass as bass
    import concourse.tile as tile
    from concourse import bacc, mybir
    from contextlib import ExitStack

    fp32 = mybir.dt.float32
    fp16 = mybir.dt.float16
    AF = mybir.ActivationFunctionType
    ALU = mybir.AluOpType

    nc = bacc.Bacc("TRN2")

    # ---- I/O (all multi-chunk tensors prepacked chunk-major on host) ----
    xTm = nc.dram_tensor("xTm", [128, HC * L], fp16, kind="ExternalInput")
    tWTm = nc.dram_tensor("tWTm", [128, HC * E], fp16, kind="ExternalInput")
    xTim = nc.dram_tensor("xTim", [128, HC * IB], fp16, kind="ExternalInput")
    sWTm = nc.dram_tensor("sWTm", [128, HC * E], fp16, kind="ExternalInput")
    WuTm = nc.dram_tensor("WuTm", [128, EC * E], fp16, kind="ExternalInput")
    WvTm = nc.dram_tensor("WvTm", [128, EC * E], fp16, kind="ExternalInput")
    # blW.T blocks ordered (out-half ec, contract-chunk epc): [128, 1536] each
    blWTm = nc.dram_tensor("blWTm", [128, 2 * E * N], fp16, kind="ExternalInput")
    f3padm = nc.dram_tensor("f3padm", [128, EC * 48], fp16, kind="ExternalInput")
    misc = nc.dram_tensor("misc", [128, MISC_W], fp32, kind="ExternalInput")
    # out[k, o, 12s+n, j] = result[8*o + 2k + s, n, j]
    out = nc.dram_tensor("out", [4, OCTS, 24, L], fp16,
                         kind="ExternalOutput")

    with tile.TileContext(nc) as tc, ExitStack() as ctx:
        consts = ctx.enter_context(tc.tile_pool(name="consts", bufs=1))
        acts = ctx.enter_context(tc.tile_pool(name="acts", bufs=1))

        # ---- PE warmup during input DMA: garbage matmuls on a memset tile
        wu = consts.tile([128, 512], fp16, name="wu")
        nc.gpsimd.memset(wu[:], 0.0)

        def load(src, shape, name, dt=fp16, eng=None):
            t = consts.tile(shape, dt, name=name)
            (eng or nc.sync).dma_start(out=t[:], in_=src)
            return t

        # queue order matters: first-needed first per queue
        tWT_m = load(tWTm[:], [128, HC * E], "tWT_m")
        xT_m = consts.tile([128, HC * L], fp16, name="xT_m")
        nc.sync.dma_start(out=xT_m[:, : 3 * L], in_=xTm[:, : 3 * L])
        nc.sync.dma_start(out=xT_m[:, 3 * L :], in_=xTm[:, 3 * L :])
        xTi_m = load(xTim[:], [128, HC * IB], "xTi_m", eng=nc.gpsimd)
        sWT_m = load(sWTm[:], [128, HC * E], "sWT_m", eng=nc.gpsimd)
        misc_sb = load(misc[:], [128, MISC_W], "misc_sb", dt=fp32, eng=nc.gpsimd)
        WvT_m = load(WvTm[:], [128, EC * E], "WvT_m", eng=nc.scalar)
        WuT_m = load(WuTm[:], [128, EC * E], "WuT_m", eng=nc.scalar)
        f3pad_m = load(f3padm[:], [128, EC * 48], "f3pad_m", eng=nc.scalar)
        blWT_m = consts.tile([128, 2 * E * N], fp16, name="blWT_m")
        nc.scalar.dma_start(out=blWT_m[:, : E * N], in_=blWTm[:, : E * N])
        nc.scalar.dma_start(out=blWT_m[:, E * N :], in_=blWTm[:, E * N :])

        xT_sb = [xT_m[:, L * c : L * (c + 1)] for c in range(HC)]
        tWT_sb = [tWT_m[:, E * c : E * (c + 1)] for c in range(HC)]
        xTi_sb = [xTi_m[:, IB * c : IB * (c + 1)] for c in range(HC)]
        sWT_sb = [sWT_m[:, E * c : E * (c + 1)] for c in range(HC)]
        WuT_sb = [WuT_m[:, E * c : E * (c + 1)] for c in range(EC)]
        WvT_sb = [WvT_m[:, E * c : E * (c + 1)] for c in range(EC)]
        # blWT block (ec, epc): [128 e'-contract, 128 e-out, 12 n]
        blk = E * N // 2  # 1536
        blWT3 = [
            [
                blWT_m[:, blk * (2 * ec + epc) : blk * (2 * ec + epc + 1)]
                .rearrange("p (e n) -> p e n", n=N)
                for epc in range(EC)
            ]
            for ec in range(EC)
        ]
        f3pad_sb = [f3pad_m[:, 48 * c : 48 * (c + 1)] for c in range(EC)]
        o_ = 0
        sb_sb = misc_sb[:, o_ : o_ + 2]; o_ += 2
        tb_sb = misc_sb[:, o_ : o_ + 2]; o_ += 2
        f2b_sb = misc_sb[:, o_ : o_ + 2]; o_ += 2
        f3b_sb = misc_sb[:, o_ : o_ + 1]; o_ += 1
        kxn01_sb = misc_sb[:, o_ : o_ + OCTS]; o_ += OCTS
        mask8_sb = misc_sb[:, o_ : o_ + 8]; o_ += 8
        f3WT_sb = [misc_sb[:, o_ + N * c : o_ + N * (c + 1)] for c in range(EC)]

        # ---- persistent activations ----
        tT_sb, sTb_sb, suT_sb, negsuT_sb, keyE_sb = [], [], [], [], []
        for ec in range(EC):
            tT_sb.append(acts.tile([128, L], fp16, name=f"tT{ec}"))
            sTb_sb.append(acts.tile([128, IB], fp16, name=f"sTb{ec}"))
            suT_sb.append(acts.tile([128, IB], fp32, name=f"suT{ec}"))
            negsuT_sb.append(acts.tile([128, IB], fp32, name=f"negsuT{ec}"))
            # key, packed: col 32*d + 12*s + n  (i = 2d+s), pads zero
            keyE_sb.append(acts.tile([128, 32 * 64], fp16, name=f"keyE_{ec}"))
            nc.gpsimd.memset(keyE_sb[ec][:], 0.0)
        tvT2c = acts.tile([128, 2 * L], fp16, name="tvT2c")  # cols 512*ec+j
        C8sb = acts.tile([128, OCTS], fp32, name="C8sb")
        CT_sb = acts.tile([128, 2 * N], fp32, name="CT_sb")  # [Csum | Cec1]
        kxmC = acts.tile([128, 128], fp32, name="kxmC")
        nc.gpsimd.memset(kxmC[:], 0.0)

        # ---- prep ----
        with tc.tile_pool(name="prep_psum", bufs=3, space="PSUM") as pp:
            # PE warmup: keep the HAM busy while inputs stream in
            ps_w = pp.tile([128, L], fp32, name="ps_w", tag="ps")
            for w in range(10):
                nc.tensor.matmul(ps_w[:], lhsT=wu[:, :128], rhs=wu[:],
                                 start=True, stop=True)

            for ec in range(EC):
                # tT = relu(x @ tW.T + tb)  (fp16 matmul, fp16 out)
                ps_t = pp.tile([128, L], fp32, name="ps_t", tag="ps")
                for hc in range(HC):
                    nc.tensor.matmul(
                        ps_t[:],
                        lhsT=tWT_sb[hc][:, 128 * ec : 128 * (ec + 1)],
                        rhs=xT_sb[hc],
                        start=(hc == 0),
                        stop=(hc == HC - 1),
                    )
                nc.scalar.activation(tT_sb[ec][:], ps_t[:], AF.Relu,
                                     bias=tb_sb[:, ec : ec + 1])

                ps_s = pp.tile([128, L], fp32, name="ps_s", tag="ps")
                for hc in range(HC):
                    nc.tensor.matmul(
                        ps_s[:, :IB],
                        lhsT=sWT_sb[hc][:, 128 * ec : 128 * (ec + 1)],
                        rhs=xTi_sb[hc],
                        start=(hc == 0),
                        stop=(hc == HC - 1),
                    )
                nc.scalar.activation(sTb_sb[ec][:], ps_s[:, :IB], AF.Relu,
                                     bias=sb_sb[:, ec : ec + 1])

            for ec in range(EC):
                # tvT chunk (fp16 matmul) -> dense fp16
                ps_tv = pp.tile([128, L], fp32, name="ps_tv", tag="ps")
                for epc in range(EC):
                    nc.tensor.matmul(
                        ps_tv[:],
                        lhsT=WvT_sb[epc][:, 128 * ec : 128 * (ec + 1)],
                        rhs=tT_sb[epc][:],
                        start=(epc == 0),
                        stop=(epc == EC - 1),
                    )
                nc.scalar.copy(tvT2c[:, L * ec : L * (ec + 1)], ps_tv[:])

                # suT = s @ Wu.T + f2b (fp16 matmul, fp32 out) and -suT
                ps_su = pp.tile([128, L], fp32, name="ps_su", tag="ps")
                for epc in range(EC):
                    nc.tensor.matmul(
                        ps_su[:, :IB],
                        lhsT=WuT_sb[epc][:, 128 * ec : 128 * (ec + 1)],
                        rhs=sTb_sb[epc][:],
                        start=(epc == 0),
                        stop=(epc == EC - 1),
                    )
                nc.scalar.activation(suT_sb[ec][:], ps_su[:, :IB], AF.Identity,
                                     bias=f2b_sb[:, ec : ec + 1])
                nc.vector.tensor_scalar_mul(negsuT_sb[ec][:], suT_sb[ec][:], -1.0)

            # correction CT[i, n] = sum_e f3WT[e,n] * suT[e,i]  (fp32)
            ps_ct = pp.tile([128, L], fp32, name="ps_ct", tag="ps")
            for ec in range(EC):
                nc.tensor.matmul(
                    ps_ct[:, :N],
                    lhsT=suT_sb[ec][:],
                    rhs=f3WT_sb[ec],
                    start=(ec == 0),
                    stop=(ec == EC - 1),
                )
            nc.vector.tensor_copy(out=CT_sb[:, :N], in_=ps_ct[:, :N])
            # kxmC[:, 32k+12s+n] = CT[:, n] * (i%8 == 2k+s), p<6 only
            for k in range(4):
                for s in range(2):
                    p = 2 * k + s
                    if p >= 6:
                        continue
                    nc.vector.tensor_tensor(
                        out=kxmC[:, 32 * k + 12 * s : 32 * k + 12 * s + N],
                        in0=CT_sb[:, :N],
                        in1=mask8_sb[:, p : p + 1].broadcast_to([128, N]),
                        op=ALU.mult,
                    )
            ps_c8 = pp.tile([128, L], fp32, name="ps_c8", tag="ps")
            nc.tensor.matmul(ps_c8[:, :OCTS], lhsT=kxmC[:], rhs=kxn01_sb,
                             start=True, stop=True)
            nc.vector.tensor_tensor(
                out=C8sb[:], in0=ps_c8[:, :OCTS],
                in1=f3b_sb.broadcast_to([128, OCTS]), op=ALU.add)

            # key (fp16 matmul): keyE[ec][e, 32d+12s+n] = key[2d+s, 128ec+e, n]
            # pairs of n in a 2-bank psum tile; one strided copy per s
            keyv = [keyE_sb[c].rearrange("p (d q) -> p d q", q=32) for c in range(EC)]
            with tc.tile_pool(name="key_psum", bufs=2, space="PSUM") as kp:
                for ec in range(EC):
                    for q in range(N // 2):
                        kp2 = kp.tile([128, 2 * L], fp32, name="kp2", tag="kp")
                        for nl in range(2):
                            for epc in range(EC):
                                nc.tensor.matmul(
                                    kp2[:, L * nl : L * nl + IB],
                                    lhsT=blWT3[ec][epc][:, :, 2 * q + nl],
                                    rhs=sTb_sb[epc][:],
                                    start=(epc == 0),
                                    stop=(epc == EC - 1),
                                )
                        # src dims (d:64 stride 2, n2:2 stride 512), offset s
                        kv = kp2[:].rearrange("p (n2 j) -> p j n2", n2=2)
                        for s in range(2):
                            dst = keyv[ec][:, :, 12 * s + 2 * q : 12 * s + 2 * q + 2]
                            if s == 0:
                                nc.vector.tensor_copy(out=dst, in_=kv[:, s : IB : 2, :])
                            else:
                                nc.scalar.copy(dst, kv[:, s : IB : 2, :])

        # ---- main loop over octets (final copy pipelined 1 octet back) ----
        hp = ctx.enter_context(tc.tile_pool(name="hp", bufs=24))
        outp = ctx.enter_context(tc.tile_pool(name="outp", bufs=2))
        mp = ctx.enter_context(tc.tile_pool(name="main_psum", bufs=5, space="PSUM"))

        pending = None  # (psum_tile, octet)
        ob4 = [None]  # current output-group tile
        # group sizes: big groups early (fewer DMA issues), small at the end
        # so the last flush only gates a small final transfer
        OGROUPS = [4, 4, 4, 2, 1, 1]
        ostart = [sum(OGROUPS[:x]) for x in range(len(OGROUPS))]

        def flush(pending):
            ps_prev, o_prev = pending
            gi = max(x for x in range(len(OGROUPS)) if ostart[x] <= o_prev)
            g = o_prev - ostart[gi]
            glen = OGROUPS[gi]
            if g == 0:
                ob4[0] = outp.tile([128, OG * L], fp16, name="ob4")
            ob = ob4[0]
            nc.scalar.activation(ob[:, L * g : L * (g + 1)], ps_prev[:],
                                 AF.Identity, bias=C8sb[:, o_prev : o_prev + 1])
            if g == glen - 1:
                for k in range(4):
                    nc.sync.dma_start(
                        out=out[k, ostart[gi] : ostart[gi] + glen]
                        .rearrange("g r j -> r g j"),
                        in_=ob[32 * k : 32 * k + 24, : glen * L]
                        .rearrange("r (g j) -> r g j", g=glen),
                    )

        # h-tile engine map: p6/p7 -> ACT (true h), p0-5 -> DVE (h')
        def h_engine(o, p, ec):
            return "act" if p >= 6 else "dve"

        for o in range(OCTS):
            ps = mp.tile([128, L], fp32, name="ps")
            # h production (engines run concurrently; PE consumes in order)
            hs = {}
            for p in (0, 1, 2, 3, 4, 5, 6, 7):
                i = 8 * o + p
                for ec in range(EC):
                    eng = h_engine(o, p, ec)
                    ht = hp.tile([128, L], fp16, name=f"h{eng}", tag="h")
                    if eng == "act":
                        nc.scalar.activation(
                            ht[:], tvT2c[:, L * ec : L * (ec + 1)],
                            AF.Relu, bias=suT_sb[ec][:, i : i + 1])
                    else:
                        nc.vector.tensor_scalar_max(
                            ht[:], tvT2c[:, L * ec : L * (ec + 1)],
                            negsuT_sb[ec][:, i : i + 1])
                    hs[(p, ec)] = ht[:]
            # out2 first: M=24; only the s=0 ec=0 matmul initializes a group
            # (s=1 shares psum rows 32k+12..23 with s=0's zero-block cols, so
            # a second start=True would wipe s=0's accumulated values)
            for ec in range(EC):
                for p in (0, 2, 4, 6, 1, 3, 7, 5):
                    k, s = divmod(p, 2)
                    nc.tensor.matmul(
                        ps[32 * k : 32 * k + 24, :],
                        lhsT=f3pad_sb[ec][:, 24 * s : 24 * s + 24],
                        rhs=hs[(p, ec)],
                        start=(ec == 0 and s == 0),
                        stop=False,
                        tile_position=(0, 32 * k),
                        skip_group_check=True,
                    )
            # out1 accumulates after: M=32 per (duo, ec), stops the bank
            for ec in range(EC):
                for k in range(4):
                    d = 4 * o + k
                    nc.tensor.matmul(
                        ps[32 * k : 32 * k + 32, :],
                        lhsT=keyE_sb[ec][:, 32 * d : 32 * d + 32],
                        rhs=tT_sb[ec][:],
                        start=False,
                        stop=(ec == EC - 1),
                        tile_position=(0, 32 * k),
                        skip_group_check=True,
                    )
            if pending is not None:
                flush(pending)
            pending = (ps, o)
        flush(pending)

    nc.compile()
    return nc


def _get_nc():
    if "nc" not in _cache:
        _cache["nc"] = build_nc()
    return _cache["nc"]


def _chunk_major(a, nchunks):
    # [128*nchunks, W] -> [128, nchunks*W] with chunk-major free layout
    W = a.shape[1]
    return np.ascontiguousarray(
        a.reshape(nchunks, 128, W).transpose(1, 0, 2).reshape(128, nchunks * W))


def _make_in_maps(inputs):
    x = np.asarray(inputs["x"], np.float32)
    f32 = lambda a: np.asarray(a, np.float32)

    f2W = f32(inputs["f2W"])
    f3WT = f32(inputs["f3W"]).T  # [E, N]
    f3pad = np.zeros((E, 48), np.float32)
    for s in range(2):
        # slice s covers psum rows 32k..32k+24; i with s=i%2 lands at +12*s
        f3pad[:, 24 * s + 12 * s : 24 * s + 12 * s + N] = f3WT

    misc = np.zeros((128, MISC_W), np.float32)
    o_ = 0
    misc[:, o_ : o_ + 2] = f32(inputs["sb"]).reshape(EC, 128).T; o_ += 2
    misc[:, o_ : o_ + 2] = f32(inputs["tb"]).reshape(EC, 128).T; o_ += 2
    misc[:, o_ : o_ + 2] = f32(inputs["f2b"]).reshape(EC, 128).T; o_ += 2
    for k in range(4):
        for s in range(2):
            misc[32 * k + 12 * s : 32 * k + 12 * s + N, o_] = f32(inputs["f3b"])
    o_ += 1
    for i in range(128):
        if i % 8 < 6:  # DVE h' positions need the C correction
            misc[i, o_ + i // 8] = 1.0
    o_ += OCTS
    for i in range(128):
        misc[i, o_ + i % 8] = 1.0
    o_ += 8
    misc[:, o_:] = _chunk_major(f3WT, EC)

    # blW.T [256 f, 3072 (e n)] -> blocks (ec out-half, epc contract-chunk)
    blWT = f32(inputs["blW"]).T
    blk_cols = E * N // 2  # 1536
    blocks = []
    for ec in range(EC):
        for epc in range(EC):
            blocks.append(blWT[128 * epc : 128 * (epc + 1),
                               blk_cols * ec : blk_cols * (ec + 1)])
    blWTm = np.concatenate(blocks, axis=1).astype(np.float16)

    shared = {
        "sWTm": _chunk_major(f32(inputs["sW"]).T, HC).astype(np.float16),
        "tWTm": _chunk_major(f32(inputs["tW"]).T, HC).astype(np.float16),
        "WuTm": _chunk_major(f2W[:, :E].T, EC).astype(np.float16),
        "WvTm": _chunk_major(f2W[:, E:].T, EC).astype(np.float16),
        "blWTm": blWTm,
        "f3padm": _chunk_major(f3pad, EC).astype(np.float16),
        "misc": misc,
    }

    in_maps = []
    for c in range(NCORES):
        b, r = divmod(c, 4)
        m = dict(shared)
        m["xTm"] = _chunk_major(
            np.ascontiguousarray(x[b].T), HC).astype(np.float16)
        m["xTim"] = _chunk_major(
            np.ascontiguousarray(x[b, IB * r : IB * (r + 1), :].T),
            HC).astype(np.float16)
        in_maps.append(m)
    return in_maps


def _gather(results):
    full = np.empty((B, L, N, L), np.float32)
    for c in range(NCORES):
        b, r = divmod(c, 4)
        # out[k, o, 12s+n, j] -> core[8*o + 2k + s, n, j]
        a = results[c]["out"].astype(np.float32)
        a = a.reshape(4, OCTS, 2, N, L)
        a = a.transpose(1, 0, 2, 3, 4).reshape(IB, N, L)
        full[b, IB * r : IB * (r + 1)] = a
    return full


def kernel(x, sW, sb, tW, tb, f2W, f2b, f3W, f3b, blW):
    from concourse.bass_utils import run_bass_kernel_spmd

    in_maps = _make_in_maps(dict(
        x=x, sW=sW, sb=sb, tW=tW, tb=tb, f2W=f2W, f2b=f2b,
        f3W=f3W, f3b=f3b, blW=blW,
    ))
    nc = _get_nc()
    res = run_bass_kernel_spmd(nc, in_maps, core_ids=list(range(NCORES)))
    return _gather(res.results)


# revision 18
# speedup vs baseline: 1.0199x; 1.0199x over previous
"""BiAffine layer kernel for 8 Trainium2 NeuronCores.

Reference computation (per batch b):
  s = relu(x @ sW.T + sb)                  [L, E]
  t = relu(x @ tW.T + tb)                  [L, E]
  key = (s @ blW.T).reshape(L, E, N)
  out1[i, n, l] = sum_e key[i, e, n] * t[l, e]
  su = s @ Wu.T ; tv = t @ Wv.T            (Wu, Wv = f2W[:, :E], f2W[:, E:])
  h[i, j, :] = relu(su[i] + tv[j] + f2b)
  out2[i, n, j] = sum_e h[i, j, e] * f3W[n, e] + f3b[n]
  out = out1 + out2                        [L, N, L]

Sharding: 8 cores = 2 batches x 4 blocks of 128 source positions (i).

Octet layout: one PSUM bank [128, 512] holds EIGHT i's: 4 col-groups at
32-aligned offsets, 2 i's packed per group (rows 32k + 12s + n, 8 pad
rows per group).  out2 runs FIRST (initializes rows :24 of each group),
out1 accumulates after and stops the bank; the 8 pad rows per group are
never DMA'd so their garbage is harmless.  This lets the key-tensor prep
(48 matmuls + 24 copies) overlap the first octets' h production.

h production per octet, position p = i%8 (tile = (p, ec)):
  DVE: p 0-5 both ec -> h' = max(tv, -su') via TENSOR_SCALAR with a
    per-partition fp32 scalar pointer (2x DVE mode; the pointer-scalar
    occupies a read port so 4x is not reachable), dense fp16 [128, 512]
    tiles.  GPSIMD cannot help: its tensor_scalar measures ~8.4us/tile.
  ACT: p6/p7 both ec -> true h = relu(tv + su') via activation bias.
The h' tiles are short by sum_e f3W[n,e]su'[e,i]; that rank-1
correction C (p<6 rows only) is computed on the PE and folded into the
final copy's per-partition bias.
Final: one ACT copy [128,512] fp16 per octet (pipelined one octet back)
into a 4-octet output buffer; 4 sync-issued DMAs per 4 octets.

PE HAM warmup: ~10 throwaway matmuls issued during the input-DMA phase
so the PE clock is ramping to 2.4 GHz when real matmuls start.

DMA-instruction issue costs ~600ns+ (HWDGE; SWDGE multi-level patterns
cost much more - avoid), so all output DMAs use sync with plain
24-partition sources, and inputs load as few fp16 chunk-major DMAs.
"""

import sys

sys.path.insert(0, "/opt/trn_rl_repo")

import numpy as np

B, L, H, E, N = 2, 512, 768, 256, 12
EC = E // 128  # 2 e-chunks
HC = H // 128  # 6 h-chunks
IB = L // 4  # 128 i's per core
NCORES = 8
OCTS = IB // 8  # 16
OG = 4  # octets per output-DMA group

# misc fp32 tensor column layout: [sb(2) tb(2) f2b(2) f3b128(1) kxn01(16)
#                                  mask8(8) f3WT(24)]
MISC_W = 2 + 2 + 2 + 1 + OCTS + 8 + 2 * N

_cache = {}


def build_nc():
    import concourse.bass as bass
    import concourse.tile as tile
    from concourse import bacc, mybir
    from contextlib import ExitStack

    fp32 = mybir.dt.float32
    fp16 = mybir.dt.float16
    AF = mybir.ActivationFunctionType
    ALU = mybir.AluOpType

    nc = bacc.Bacc("TRN2")

    # ---- I/O (all multi-chunk tensors prepacked chunk-major on host) ----
    xTm = nc.dram_tensor("xTm", [128, HC * L], fp16, kind="ExternalInput")
    tWTm = nc.dram_tensor("tWTm", [128, HC * E], fp16, kind="ExternalInput")
    xTim = nc.dram_tensor("xTim", [128, HC * IB], fp16, kind="ExternalInput")
    sWTm = nc.dram_tensor("sWTm", [128, HC * E], fp16, kind="ExternalInput")
    WuTm = nc.dram_tensor("WuTm", [128, EC * E], fp16, kind="ExternalInput")
    WvTm = nc.dram_tensor("WvTm", [128, EC * E], fp16, kind="ExternalInput")
    # blW.T blocks ordered (out-half ec, contract-chunk epc): [128, 1536] each
    blWTm = nc.dram_tensor("blWTm", [128, 2 * E * N], fp16, kind="ExternalInput")
    f3padm = nc.dram_tensor("f3padm", [128, EC * 48], fp16, kind="ExternalInput")
    misc = nc.dram_tensor("misc", [128, MISC_W], fp32, kind="ExternalInput")
    # out[k, G, g, 12s+n, j] = result[8*(OG*G+g) + 2k + s, n, j]
    out = nc.dram_tensor("out", [4, OCTS // OG, OG, 24, L], fp16,
                         kind="ExternalOutput")

    with tile.TileContext(nc) as tc, ExitStack() as ctx:
        consts = ctx.enter_context(tc.tile_pool(name="consts", bufs=1))
        acts = ctx.enter_context(tc.tile_pool(name="acts", bufs=1))

        # ---- PE warmup during input DMA: garbage matmuls on a memset tile
        wu = consts.tile([128, 512], fp16, name="wu")
        nc.gpsimd.memset(wu[:], 0.0)

        def load(src, shape, name, dt=fp16, eng=None):
            t = consts.tile(shape, dt, name=name)
            (eng or nc.sync).dma_start(out=t[:], in_=src)
            return t

        # queue order matters: first-needed first per queue
        tWT_m = load(tWTm[:], [128, HC * E], "tWT_m")
        xT_m = consts.tile([128, HC * L], fp16, name="xT_m")
        nc.sync.dma_start(out=xT_m[:, : 3 * L], in_=xTm[:, : 3 * L])
        nc.sync.dma_start(out=xT_m[:, 3 * L :], in_=xTm[:, 3 * L :])
        xTi_m = load(xTim[:], [128, HC * IB], "xTi_m", eng=nc.gpsimd)
        sWT_m = load(sWTm[:], [128, HC * E], "sWT_m", eng=nc.gpsimd)
        misc_sb = load(misc[:], [128, MISC_W], "misc_sb", dt=fp32, eng=nc.gpsimd)
        WvT_m = load(WvTm[:], [128, EC * E], "WvT_m", eng=nc.scalar)
        WuT_m = load(WuTm[:], [128, EC * E], "WuT_m", eng=nc.scalar)
        f3pad_m = load(f3padm[:], [128, EC * 48], "f3pad_m", eng=nc.scalar)
        blWT_m = consts.tile([128, 2 * E * N], fp16, name="blWT_m")
        nc.scalar.dma_start(out=blWT_m[:, : E * N], in_=blWTm[:, : E * N])
        nc.scalar.dma_start(out=blWT_m[:, E * N :], in_=blWTm[:, E * N :])

        xT_sb = [xT_m[:, L * c : L * (c + 1)] for c in range(HC)]
        tWT_sb = [tWT_m[:, E * c : E * (c + 1)] for c in range(HC)]
        xTi_sb = [xTi_m[:, IB * c : IB * (c + 1)] for c in range(HC)]
        sWT_sb = [sWT_m[:, E * c : E * (c + 1)] for c in range(HC)]
        WuT_sb = [WuT_m[:, E * c : E * (c + 1)] for c in range(EC)]
        WvT_sb = [WvT_m[:, E * c : E * (c + 1)] for c in range(EC)]
        # blWT block (ec, epc): [128 e'-contract, 128 e-out, 12 n]
        blk = E * N // 2  # 1536
        blWT3 = [
            [
                blWT_m[:, blk * (2 * ec + epc) : blk * (2 * ec + epc + 1)]
                .rearrange("p (e n) -> p e n", n=N)
                for epc in range(EC)
            ]
            for ec in range(EC)
        ]
        f3pad_sb = [f3pad_m[:, 48 * c : 48 * (c + 1)] for c in range(EC)]
        o_ = 0
        sb_sb = misc_sb[:, o_ : o_ + 2]; o_ += 2
        tb_sb = misc_sb[:, o_ : o_ + 2]; o_ += 2
        f2b_sb = misc_sb[:, o_ : o_ + 2]; o_ += 2
        f3b_sb = misc_sb[:, o_ : o_ + 1]; o_ += 1
        kxn01_sb = misc_sb[:, o_ : o_ + OCTS]; o_ += OCTS
        mask8_sb = misc_sb[:, o_ : o_ + 8]; o_ += 8
        f3WT_sb = [misc_sb[:, o_ + N * c : o_ + N * (c + 1)] for c in range(EC)]

        # ---- persistent activations ----
        tT_sb, sTb_sb, suT_sb, negsuT_sb, keyE_sb = [], [], [], [], []
        for ec in range(EC):
            tT_sb.append(acts.tile([128, L], fp16, name=f"tT{ec}"))
            sTb_sb.append(acts.tile([128, IB], fp16, name=f"sTb{ec}"))
            suT_sb.append(acts.tile([128, IB], fp32, name=f"suT{ec}"))
            negsuT_sb.append(acts.tile([128, IB], fp32, name=f"negsuT{ec}"))
            # key, packed: col 32*d + 12*s + n  (i = 2d+s), pads zero
            keyE_sb.append(acts.tile([128, 32 * 64], fp16, name=f"keyE_{ec}"))
            nc.gpsimd.memset(keyE_sb[ec][:], 0.0)
        tvT2c = acts.tile([128, 2 * L], fp16, name="tvT2c")  # cols 512*ec+j
        C8sb = acts.tile([128, OCTS], fp32, name="C8sb")
        CT_sb = acts.tile([128, 2 * N], fp32, name="CT_sb")  # [Csum | Cec1]
        kxmC = acts.tile([128, 128], fp32, name="kxmC")
        nc.gpsimd.memset(kxmC[:], 0.0)

        # ---- prep ----
        with tc.tile_pool(name="prep_psum", bufs=3, space="PSUM") as pp:
            # PE warmup: keep the HAM busy while inputs stream in
            ps_w = pp.tile([128, L], fp32, name="ps_w", tag="ps")
            for w in range(10):
                nc.tensor.matmul(ps_w[:], lhsT=wu[:, :128], rhs=wu[:],
                                 start=True, stop=True)

            for ec in range(EC):
                # tT = relu(x @ tW.T + tb)  (fp16 matmul, fp16 out)
                ps_t = pp.tile([128, L], fp32, name="ps_t", tag="ps")
                for hc in range(HC):
                    nc.tensor.matmul(
                        ps_t[:],
                        lhsT=tWT_sb[hc][:, 128 * ec : 128 * (ec + 1)],
                        rhs=xT_sb[hc],
                        start=(hc == 0),
                        stop=(hc == HC - 1),
                    )
                nc.scalar.activation(tT_sb[ec][:], ps_t[:], AF.Relu,
                                     bias=tb_sb[:, ec : ec + 1])

                ps_s = pp.tile([128, L], fp32, name="ps_s", tag="ps")
                for hc in range(HC):
                    nc.tensor.matmul(
                        ps_s[:, :IB],
                        lhsT=sWT_sb[hc][:, 128 * ec : 128 * (ec + 1)],
                        rhs=xTi_sb[hc],
                        start=(hc == 0),
                        stop=(hc == HC - 1),
                    )
                nc.scalar.activation(sTb_sb[ec][:], ps_s[:, :IB], AF.Relu,
                                     bias=sb_sb[:, ec : ec + 1])

            for ec in range(EC):
                # tvT chunk (fp16 matmul) -> dense fp16
                ps_tv = pp.tile([128, L], fp32, name="ps_tv", tag="ps")
                for epc in range(EC):
                    nc.tensor.matmul(
                        ps_tv[:],
                        lhsT=WvT_sb[epc][:, 128 * ec : 128 * (ec + 1)],
                        rhs=tT_sb[epc][:],
                        start=(epc == 0),
                        stop=(epc == EC - 1),
                    )
                nc.scalar.copy(tvT2c[:, L * ec : L * (ec + 1)], ps_tv[:])

                # suT = s @ Wu.T + f2b (fp16 matmul, fp32 out) and -suT
                ps_su = pp.tile([128, L], fp32, name="ps_su", tag="ps")
                for epc in range(EC):
                    nc.tensor.matmul(
                        ps_su[:, :IB],
                        lhsT=WuT_sb[epc][:, 128 * ec : 128 * (ec + 1)],
                        rhs=sTb_sb[epc][:],
                        start=(epc == 0),
                        stop=(epc == EC - 1),
                    )
                nc.scalar.activation(suT_sb[ec][:], ps_su[:, :IB], AF.Identity,
                                     bias=f2b_sb[:, ec : ec + 1])
                nc.vector.tensor_scalar_mul(negsuT_sb[ec][:], suT_sb[ec][:], -1.0)

            # correction CT[i, n] = sum_e f3WT[e,n] * suT[e,i]  (fp32)
            ps_ct = pp.tile([128, L], fp32, name="ps_ct", tag="ps")
            for ec in range(EC):
                nc.tensor.matmul(
                    ps_ct[:, :N],
                    lhsT=suT_sb[ec][:],
                    rhs=f3WT_sb[ec],
                    start=(ec == 0),
                    stop=(ec == EC - 1),
                )
            nc.vector.tensor_copy(out=CT_sb[:, :N], in_=ps_ct[:, :N])
            # kxmC[:, 32k+12s+n] = CT[:, n] * (i%8 == 2k+s), p<6 only
            for k in range(4):
                for s in range(2):
                    p = 2 * k + s
                    if p >= 6:
                        continue
                    nc.vector.tensor_tensor(
                        out=kxmC[:, 32 * k + 12 * s : 32 * k + 12 * s + N],
                        in0=CT_sb[:, :N],
                        in1=mask8_sb[:, p : p + 1].broadcast_to([128, N]),
                        op=ALU.mult,
                    )
            ps_c8 = pp.tile([128, L], fp32, name="ps_c8", tag="ps")
            nc.tensor.matmul(ps_c8[:, :OCTS], lhsT=kxmC[:], rhs=kxn01_sb,
                             start=True, stop=True)
            nc.vector.tensor_tensor(
                out=C8sb[:], in0=ps_c8[:, :OCTS],
                in1=f3b_sb.broadcast_to([128, OCTS]), op=ALU.add)

            # key (fp16 matmul): keyE[ec][e, 32d+12s+n] = key[2d+s, 128ec+e, n]
            # pairs of n in a 2-bank psum tile; one strided copy per s
            keyv = [keyE_sb[c].rearrange("p (d q) -> p d q", q=32) for c in range(EC)]
            with tc.tile_pool(name="key_psum", bufs=2, space="PSUM") as kp:
                for ec in range(EC):
                    for q in range(N // 2):
                        kp2 = kp.tile([128, 2 * L], fp32, name="kp2", tag="kp")
                        for nl in range(2):
                            for epc in range(EC):
                                nc.tensor.matmul(
                                    kp2[:, L * nl : L * nl + IB],
                                    lhsT=blWT3[ec][epc][:, :, 2 * q + nl],
                                    rhs=sTb_sb[epc][:],
                                    start=(epc == 0),
                                    stop=(epc == EC - 1),
                                )
                        # src dims (d:64 stride 2, n2:2 stride 512), offset s
                        kv = kp2[:].rearrange("p (n2 j) -> p j n2", n2=2)
                        for s in range(2):
                            dst = keyv[ec][:, :, 12 * s + 2 * q : 12 * s + 2 * q + 2]
                            if s == 0:
                                nc.vector.tensor_copy(out=dst, in_=kv[:, s : IB : 2, :])
                            else:
                                nc.scalar.copy(dst, kv[:, s : IB : 2, :])

        # ---- main loop over octets (final copy pipelined 1 octet back) ----
        hp = ctx.enter_context(tc.tile_pool(name="hp", bufs=24))
        outp = ctx.enter_context(tc.tile_pool(name="outp", bufs=2))
        mp = ctx.enter_context(tc.tile_pool(name="main_psum", bufs=5, space="PSUM"))

        pending = None  # (psum_tile, octet)
        ob4 = [None]  # current output-group tile

        def flush(pending):
            ps_prev, o_prev = pending
            g = o_prev % OG
            if g == 0:
                ob4[0] = outp.tile([128, OG * L], fp16, name="ob4")
            ob = ob4[0]
            nc.scalar.activation(ob[:, L * g : L * (g + 1)], ps_prev[:],
                                 AF.Identity, bias=C8sb[:, o_prev : o_prev + 1])
            if g == OG - 1:
                G = o_prev // OG
                for k in range(4):
                    nc.sync.dma_start(
                        out=out[k, G].rearrange("g r j -> r g j"),
                        in_=ob[32 * k : 32 * k + 24, :]
                        .rearrange("r (g j) -> r g j", g=OG),
                    )

        # h-tile engine map: p6/p7 -> ACT (true h), p0-5 -> DVE (h')
        def h_engine(o, p, ec):
            return "act" if p >= 6 else "dve"

        for o in range(OCTS):
            ps = mp.tile([128, L], fp32, name="ps")
            # h production (engines run concurrently; PE consumes in order)
            hs = {}
            for p in (0, 1, 2, 3, 4, 5, 6, 7):
                i = 8 * o + p
                for ec in range(EC):
                    eng = h_engine(o, p, ec)
                    ht = hp.tile([128, L], fp16, name=f"h{eng}", tag="h")
                    if eng == "act":
                        nc.scalar.activation(
                            ht[:], tvT2c[:, L * ec : L * (ec + 1)],
                            AF.Relu, bias=suT_sb[ec][:, i : i + 1])
                    else:
                        nc.vector.tensor_scalar_max(
                            ht[:], tvT2c[:, L * ec : L * (ec + 1)],
                            negsuT_sb[ec][:, i : i + 1])
                    hs[(p, ec)] = ht[:]
            # out2 first: M=24; only the s=0 ec=0 matmul initializes a group
            # (s=1 shares psum rows 32k+12..23 with s=0's zero-block cols, so
            # a second start=True would wipe s=0's accumulated values)
            for ec in range(EC):
                for p in (0, 2, 4, 6, 1, 3, 7, 5):
                    k, s = divmod(p, 2)
                    nc.tensor.matmul(
                        ps[32 * k : 32 * k + 24, :],
                        lhsT=f3pad_sb[ec][:, 24 * s : 24 * s + 24],
                        rhs=hs[(p, ec)],
                        start=(ec == 0 and s == 0),
                        stop=False,
                        tile_position=(0, 32 * k),
                        skip_group_check=True,
                    )
            # out1 accumulates after: M=32 per (duo, ec), stops the bank
            for ec in range(EC):
                for k in range(4):
                    d = 4 * o + k
                    nc.tensor.matmul(
                        ps[32 * k : 32 * k + 32, :],
                        lhsT=keyE_sb[ec][:, 32 * d : 32 * d + 32],
                        rhs=tT_sb[ec][:],
                        start=False,
                        stop=(ec == EC - 1),
                        tile_position=(0, 32 * k),
                        skip_group_check=True,
                    )
            if pending is not None:
                flush(pending)
            pending = (ps, o)
        flush(pending)

    nc.compile()
    return nc


def _get_nc():
    if "nc" not in _cache:
        _cache["nc"] = build_nc()
    return _cache["nc"]


def _chunk_major(a, nchunks):
    # [128*nchunks, W] -> [128, nchunks*W] with chunk-major free layout
    W = a.shape[1]
    return np.ascontiguousarray(
        a.reshape(nchunks, 128, W).transpose(1, 0, 2).reshape(128, nchunks * W))


def _make_in_maps(inputs):
    x = np.asarray(inputs["x"], np.float32)
    f32 = lambda a: np.asarray(a, np.float32)

    f2W = f32(inputs["f2W"])
    f3WT = f32(inputs["f3W"]).T  # [E, N]
    f3pad = np.zeros((E, 48), np.float32)
    for s in range(2):
        # slice s covers psum rows 32k..32k+24; i with s=i%2 lands at +12*s
        f3pad[:, 24 * s + 12 * s : 24 * s + 12 * s + N] = f3WT

    misc = np.zeros((128, MISC_W), np.float32)
    o_ = 0
    misc[:, o_ : o_ + 2] = f32(inputs["sb"]).reshape(EC, 128).T; o_ += 2
    misc[:, o_ : o_ + 2] = f32(inputs["tb"]).reshape(EC, 128).T; o_ += 2
    misc[:, o_ : o_ + 2] = f32(inputs["f2b"]).reshape(EC, 128).T; o_ += 2
    for k in range(4):
        for s in range(2):
            misc[32 * k + 12 * s : 32 * k + 12 * s + N, o_] = f32(inputs["f3b"])
    o_ += 1
    for i in range(128):
        if i % 8 < 6:  # DVE h' positions need the C correction
            misc[i, o_ + i // 8] = 1.0
    o_ += OCTS
    for i in range(128):
        misc[i, o_ + i % 8] = 1.0
    o_ += 8
    misc[:, o_:] = _chunk_major(f3WT, EC)

    # blW.T [256 f, 3072 (e n)] -> blocks (ec out-half, epc contract-chunk)
    blWT = f32(inputs["blW"]).T
    blk_cols = E * N // 2  # 1536
    blocks = []
    for ec in range(EC):
        for epc in range(EC):
            blocks.append(blWT[128 * epc : 128 * (epc + 1),
                               blk_cols * ec : blk_cols * (ec + 1)])
    blWTm = np.concatenate(blocks, axis=1).astype(np.float16)

    shared = {
        "sWTm": _chunk_major(f32(inputs["sW"]).T, HC).astype(np.float16),
        "tWTm": _chunk_major(f32(inputs["tW"]).T, HC).astype(np.float16),
        "WuTm": _chunk_major(f2W[:, :E].T, EC).astype(np.float16),
        "WvTm": _chunk_major(f2W[:, E:].T, EC).astype(np.float16),
        "blWTm": blWTm,
        "f3padm": _chunk_major(f3pad, EC).astype(np.float16),
        "misc": misc,
    }

    in_maps = []
    for c in range(NCORES):
        b, r = divmod(c, 4)
        m = dict(shared)
        m["xTm"] = _chunk_major(
            np.ascontiguousarray(x[b].T), HC).astype(np.float16)
        m["xTim"] = _chunk_major(
            np.ascontiguousarray(x[b, IB * r : IB * (r + 1), :].T),
            HC).astype(np.float16)
        in_maps.append(m)
    return in_maps


def _gather(results):
    full = np.empty((B, L, N, L), np.float32)
    for c in range(NCORES):
        b, r = divmod(c, 4)
        # out[k, G, g, 12s+n, j] -> core[8*(OG*G+g) + 2k + s, n, j]
        a = results[c]["out"].astype(np.float32)
        a = a.reshape(4, OCTS // OG, OG, 2, N, L)
        a = a.transpose(1, 2, 0, 3, 4, 5).reshape(IB, N, L)
        full[b, IB * r : IB * (r + 1)] = a
    return full


def kernel(x, sW, sb, tW, tb, f2W, f2b, f3W, f3b, blW):
    from concourse.bass_utils import run_bass_kernel_spmd

    in_maps = _make_in_maps(dict(
        x=x, sW=sW, sb=sb, tW=tW, tb=tb, f2W=f2W, f2b=f2b,
        f3W=f3W, f3b=f3b, blW=blW,
    ))
    nc = _get_nc()
    res = run_bass_kernel_spmd(nc, in_maps, core_ids=list(range(NCORES)))
    return _gather(res.results)
